# revision 16
# baseline (speedup 1.0000x reference)
"""MithralNN (PQ/vq_codebook) Trainium2 kernel.

out[n, m] = sum_c luts[c, argmin_k ||X[n, cS:(c+1)S] - protos[c,k]||^2, m] * mean(X)

Strategy (8 NeuronCores, data-parallel over rows of X):
  host:   transpose each core's X shard to [D, NL] and split into fp16
          hi/lo halves (same total bytes as fp32, but PE streams fp16 at
          2x the rate); same hi/lo split for the block-diagonal -2*protos
          weights; ||p||^2 bias as a per-partition column; luts scaled by
          mean(X) and cast to fp16.
  device, per 512-row group:
          dists^T[ck, n] = sum of 3 fp16 hi/lo cross-product matmuls
          (= fp32-accurate dots; error ~2^-22)        (PE)
          += ||p||^2 via per-partition tensor_scalar add -> SBUF  (DVE)
          DVE 32x32 stream-transpose puts k on the free dim: segmented
          min-reduce + is_equal give the one-hot in block-transposed space,
          and a second stream-transpose yields onehot^T[ck, n] directly.
          out = onehot @ luts (PE, fp16), out copy on ScalarE, DMA out.
"""

import os
import sys
import types

import numpy as np

import concourse.bacc as bacc
import concourse.mybir as mybir
import concourse.tile as tile
from concourse.bass_utils import run_bass_kernel_spmd

F32 = mybir.dt.float32
F16 = mybir.dt.float16

N, D = 32768, 512
C, K, S = 16, 16, 32
CK = C * K          # 256
M = 1024
NCORES = 8
NL = N // NCORES    # 4096 rows per core
NT = NL // 128      # 32 row tiles per core
NG = NL // 512      # 8 chunks of 512 rows


def _install_profile_shim():
    try:
        if "antenv.axon_hooks" not in sys.modules:
            import antenv

            mod = types.ModuleType("antenv.axon_hooks")
            mod._hook = None
            mod.set_axon_ntff_profile_hook = lambda h: setattr(mod, "_hook", h)
            mod.get_axon_ntff_profile_hook = lambda: mod._hook
            sys.modules["antenv.axon_hooks"] = mod
            antenv.axon_hooks = mod
            from trn_agent_boot.trn_boot import _ntff_profile_via_ctypes

            mod.set_axon_ntff_profile_hook(
                _ntff_profile_via_ctypes("/opt/axon/libaxon_pjrt.so")
            )
        return True
    except Exception:
        return False


def _build_program():
    nc = bacc.Bacc("TRN2", target_bir_lowering=False, debug=False)

    xth = nc.dram_tensor("xth", [D, NL], F16, kind="ExternalInput")
    xtl = nc.dram_tensor("xtl", [D, NL], F16, kind="ExternalInput")
    w4 = nc.dram_tensor("w4", [128, 512], F16, kind="ExternalInput")
    pnt = nc.dram_tensor("pnt", [128, 2], F32, kind="ExternalInput")
    luts = nc.dram_tensor("luts", [128, 2 * M], F16, kind="ExternalInput")
    out = nc.dram_tensor("out", [NL, M], F32, kind="ExternalOutput")

    add = mybir.AluOpType.add
    amin = mybir.AluOpType.min
    iseq = mybir.AluOpType.is_equal
    ax_x = mybir.AxisListType.X

    with tile.TileContext(nc) as tc:
        with (
            tc.tile_pool(name="const", bufs=1) as constp,
            tc.tile_pool(name="xch", bufs=3) as xpool,
            tc.tile_pool(name="work", bufs=4) as workp,
            tc.tile_pool(name="osb", bufs=3) as osbp,
            tc.tile_pool(name="pdT", bufs=2, space="PSUM") as pdT,
            tc.tile_pool(name="pout", bufs=3, space="PSUM") as pout,
        ):
            w4sb = constp.tile([128, 512], F16)
            nc.sync.dma_start(w4sb[:], w4[:])
            pnsb = constp.tile([128, 2], F32)
            nc.sync.dma_start(pnsb[:], pnt[:])
            lutsb = constp.tile([128, 2 * M], F16)
            nc.sync.dma_start(lutsb[:], luts[:])

            for g in range(NG):
                cjh = xpool.tile([128, 4 * 512], F16, tag="xh", name=f"cjh{g}")
                cjl = xpool.tile([128, 4 * 512], F16, tag="xl", name=f"cjl{g}")
                for b in range(4):
                    nc.sync.dma_start(
                        cjh[:, 512 * b : 512 * (b + 1)],
                        xth[128 * b : 128 * (b + 1), 512 * g : 512 * (g + 1)],
                    )
                    nc.sync.dma_start(
                        cjl[:, 512 * b : 512 * (b + 1)],
                        xtl[128 * b : 128 * (b + 1), 512 * g : 512 * (g + 1)],
                    )
                # --- encode: dists'^T[ck, n] = sum_d (-2W)[d,ck] * xt[d,n]
                # fp16 hi/lo cross products: Whi*Xhi + Whi*Xlo + Wlo*Xhi
                dT = [
                    pdT.tile([128, 512], F32, tag="dT", name=f"dT{g}_{h}")
                    for h in range(2)
                ]
                for b in range(4):
                    dst = dT[b // 2][64 * (b % 2) : 64 * (b % 2) + 64, :]
                    whi = w4sb[:, 64 * b : 64 * (b + 1)]
                    wlo = w4sb[:, 256 + 64 * b : 256 + 64 * (b + 1)]
                    xh_ = cjh[:, 512 * b : 512 * (b + 1)]
                    xl_ = cjl[:, 512 * b : 512 * (b + 1)]
                    nc.tensor.matmul(dst, lhsT=whi, rhs=xh_, start=True, stop=False)
                    nc.tensor.matmul(dst, lhsT=whi, rhs=xl_, start=False, stop=False)
                    nc.tensor.matmul(dst, lhsT=wlo, rhs=xh_, start=False, stop=True)
                otH = []
                for h in range(2):
                    # --- += ||p||^2 (per-partition scalar), PSUM -> SBUF
                    s = workp.tile([128, 512], F32, tag="dstT", name=f"dstT{g}_{h}")
                    nc.vector.tensor_scalar(
                        s[:], dT[h][:], pnsb[:, h : h + 1], None, op0=add
                    )
                    # --- 32x32 block transpose: k moves to the free dim.
                    # B[32P+i, 32F+j] = dists[n0+32F+i, 128h+32P+j]
                    bt = workp.tile([128, 512], F32, tag="B", name=f"B{g}_{h}")
                    nc.vector.transpose(bt[:], s[:])
                    b4 = bt.rearrange("p (f c2 k) -> p f c2 k", c2=2, k=K)
                    # --- min over k, one-hot (in block-transposed space)
                    mb = workp.tile([128, 32], F32, tag="minB", name=f"mB{g}_{h}")
                    nc.vector.tensor_reduce(
                        mb.rearrange("p (f c2) -> p f c2", c2=2), b4, axis=ax_x,
                        op=amin,
                    )
                    ohb = workp.tile([128, 512], F16, tag="ohB", name=f"oB{g}_{h}")
                    nc.vector.tensor_tensor(
                        ohb.rearrange("p (f c2 k) -> p f c2 k", c2=2, k=K),
                        b4,
                        mb.rearrange("p (f c2) -> p f c2", c2=2)[
                            :, :, :, None
                        ].broadcast_to([128, 16, 2, K]),
                        op=iseq,
                    )
                    # --- transpose back: otH[p, f] = onehot[n0+f, 128h+p]
                    o = workp.tile([128, 512], F16, tag="otH", name=f"oT{g}_{h}")
                    nc.vector.transpose(o[:], ohb[:])
                    otH.append(o)
                for tt in range(4):
                    t = 4 * g + tt
                    # --- LUT gather-accumulate: out[n, m] = onehot @ luts
                    ops = pout.tile([128, M], F32, tag="ops", name=f"ops{t}")
                    for h in range(2):
                        for mh in range(2):
                            nc.tensor.matmul(
                                ops[:, 512 * mh : 512 * (mh + 1)],
                                lhsT=otH[h][:, 128 * tt : 128 * (tt + 1)],
                                rhs=lutsb[
                                    :, M * h + 512 * mh : M * h + 512 * (mh + 1)
                                ],
                                start=(h == 0),
                                stop=(h == 1),
                            )
                    osb = osbp.tile([128, M], F32, tag="osb", name=f"osb{t}")
                    nc.scalar.copy(osb[:], ops[:])
                    nc.sync.dma_start(out[128 * t : 128 * (t + 1), :], osb[:])

    nc.compile()
    return nc


_CACHE = {}


def _prep_shared(protos: np.ndarray, luts: np.ndarray, mean: float):
    wfull = np.zeros((D, CK), dtype=np.float32)
    for c in range(C):
        wfull[S * c : S * (c + 1), K * c : K * (c + 1)] = -2.0 * protos[c].T
    w4h = np.concatenate(
        [wfull[128 * b : 128 * (b + 1), 64 * b : 64 * (b + 1)] for b in range(4)],
        axis=1,
    )  # [128, 256] fp32
    whi = w4h.astype(np.float16)
    wlo = (w4h - whi.astype(np.float32)).astype(np.float16)
    w4hl = np.ascontiguousarray(np.concatenate([whi, wlo], axis=1))  # [128, 512]
    pnorm = (protos.astype(np.float64) ** 2).sum(-1).astype(np.float32).reshape(CK)
    pnt = np.ascontiguousarray(np.stack([pnorm[:128], pnorm[128:]], axis=1))
    lf = (luts.reshape(CK, M).astype(np.float64) * mean).astype(np.float32)
    luts2 = np.ascontiguousarray(
        np.concatenate([lf[:128], lf[128:]], axis=1)
    ).astype(np.float16)
    return w4hl, pnt, luts2


def kernel(X: np.ndarray, protos: np.ndarray, luts: np.ndarray) -> np.ndarray:
    X = np.asarray(X, dtype=np.float32)
    protos = np.asarray(protos, dtype=np.float32)
    luts = np.asarray(luts, dtype=np.float32)

    mean = float(np.mean(X, dtype=np.float64))
    w4hl, pnt, luts2 = _prep_shared(protos, luts, mean)

    if "nc" not in _CACHE:
        _CACHE["nc"] = _build_program()
    nc = _CACHE["nc"]

    in_maps = []
    for i in range(NCORES):
        xt = np.ascontiguousarray(X[NL * i : NL * (i + 1)].T)  # [D, NL] fp32
        xh = xt.astype(np.float16)
        xl = (xt - xh.astype(np.float32)).astype(np.float16)
        in_maps.append(
            {
                "xth": np.ascontiguousarray(xh),
                "xtl": np.ascontiguousarray(xl),
                "w4": w4hl,
                "pnt": pnt,
                "luts": luts2,
            }
        )

    trace = bool(os.environ.get("VQ_TRACE"))
    if trace:
        trace = _install_profile_shim()

    res = run_bass_kernel_spmd(nc, in_maps, core_ids=list(range(NCORES)), trace=trace)
    _CACHE["exec_time_ns"] = res.exec_time_ns
    _CACHE["profile_json"] = res.profile_json

    return np.concatenate([res.results[i]["out"] for i in range(NCORES)], axis=0)


# revision 17
# speedup vs baseline: 1.2237x; 1.2237x over previous
"""MithralNN (PQ/vq_codebook) Trainium2 kernel.

out[n, m] = sum_c luts[c, argmin_k ||X[n, cS:(c+1)S] - protos[c,k]||^2, m] * mean(X)

Strategy (8 NeuronCores, data-parallel over rows of X):
  host:   transpose each core's X shard to [D, NL] and split into fp16
          hi/lo halves (same total bytes as fp32, but PE streams fp16 at
          2x the rate); same hi/lo split for the block-diagonal -2*protos
          weights; ||p||^2 bias as a per-partition column; luts scaled by
          mean(X) and cast to fp16.
  device, per 512-row group:
          dists^T[ck, n] = sum of 3 fp16 hi/lo cross-product matmuls
          (= fp32-accurate dots; error ~2^-22)        (PE)
          += ||p||^2 via per-partition tensor_scalar add -> SBUF  (DVE)
          DVE 32x32 stream-transpose puts k on the free dim: segmented
          min-reduce + is_equal give the one-hot in block-transposed space,
          and a second stream-transpose yields onehot^T[ck, n] directly.
          out = onehot @ luts (PE, fp16), out copy on ScalarE, DMA out.
"""

import os
import sys
import types

import numpy as np

import concourse.bacc as bacc
import concourse.mybir as mybir
import concourse.tile as tile
from concourse.bass_utils import run_bass_kernel_spmd

F32 = mybir.dt.float32
F16 = mybir.dt.float16

N, D = 32768, 512
C, K, S = 16, 16, 32
CK = C * K          # 256
M = 1024
NCORES = 8
NL = N // NCORES    # 4096 rows per core
NT = NL // 128      # 32 row tiles per core
NG = NL // 512      # 8 chunks of 512 rows


def _install_profile_shim():
    try:
        if "antenv.axon_hooks" not in sys.modules:
            import antenv

            mod = types.ModuleType("antenv.axon_hooks")
            mod._hook = None
            mod.set_axon_ntff_profile_hook = lambda h: setattr(mod, "_hook", h)
            mod.get_axon_ntff_profile_hook = lambda: mod._hook
            sys.modules["antenv.axon_hooks"] = mod
            antenv.axon_hooks = mod
            from trn_agent_boot.trn_boot import _ntff_profile_via_ctypes

            mod.set_axon_ntff_profile_hook(
                _ntff_profile_via_ctypes("/opt/axon/libaxon_pjrt.so")
            )
        return True
    except Exception:
        return False


def _build_program():
    nc = bacc.Bacc("TRN2", target_bir_lowering=False, debug=False)

    xthl = nc.dram_tensor("xthl", [D, NG, 1024], F16, kind="ExternalInput")
    w4 = nc.dram_tensor("w4", [128, 512], F16, kind="ExternalInput")
    pnt = nc.dram_tensor("pnt", [128, 2], F32, kind="ExternalInput")
    luts = nc.dram_tensor("luts", [128, 2 * M], F16, kind="ExternalInput")
    out = nc.dram_tensor("out", [NL, M], F32, kind="ExternalOutput")

    add = mybir.AluOpType.add
    amin = mybir.AluOpType.min
    iseq = mybir.AluOpType.is_equal
    ax_x = mybir.AxisListType.X

    with tile.TileContext(nc) as tc:
        with (
            tc.tile_pool(name="const", bufs=1) as constp,
            tc.tile_pool(name="xch", bufs=3) as xpool,
            tc.tile_pool(name="work", bufs=4) as workp,
            tc.tile_pool(name="osb", bufs=3) as osbp,
            tc.tile_pool(name="pdT", bufs=2, space="PSUM") as pdT,
            tc.tile_pool(name="pout", bufs=3, space="PSUM") as pout,
        ):
            w4sb = constp.tile([128, 512], F16)
            nc.sync.dma_start(w4sb[:], w4[:])
            pnsb = constp.tile([128, 2], F32)
            nc.sync.dma_start(pnsb[:], pnt[:])
            lutsb = constp.tile([128, 2 * M], F16)
            nc.sync.dma_start(lutsb[:], luts[:])

            for g in range(NG):
                cj = xpool.tile([128, 4 * 1024], F16, tag="xhl", name=f"cj{g}")
                for b in range(4):
                    nc.sync.dma_start(
                        cj[:, 1024 * b : 1024 * (b + 1)],
                        xthl[128 * b : 128 * (b + 1), g, :],
                    )
                # --- encode: dists'^T[ck, n] = sum_d (-2W)[d,ck] * xt[d,n]
                # fp16 hi/lo cross products: Whi*Xhi + Whi*Xlo + Wlo*Xhi
                dT = [
                    pdT.tile([128, 512], F32, tag="dT", name=f"dT{g}_{h}")
                    for h in range(2)
                ]
                for b in range(4):
                    dst = dT[b // 2][64 * (b % 2) : 64 * (b % 2) + 64, :]
                    whi = w4sb[:, 64 * b : 64 * (b + 1)]
                    wlo = w4sb[:, 256 + 64 * b : 256 + 64 * (b + 1)]
                    xh_ = cj[:, 1024 * b : 1024 * b + 512]
                    xl_ = cj[:, 1024 * b + 512 : 1024 * b + 1024]
                    nc.tensor.matmul(dst, lhsT=whi, rhs=xh_, start=True, stop=False)
                    nc.tensor.matmul(dst, lhsT=whi, rhs=xl_, start=False, stop=False)
                    nc.tensor.matmul(dst, lhsT=wlo, rhs=xh_, start=False, stop=True)
                otH = []
                for h in range(2):
                    # --- += ||p||^2 (per-partition scalar), PSUM -> SBUF
                    s = workp.tile([128, 512], F32, tag="dstT", name=f"dstT{g}_{h}")
                    nc.vector.tensor_scalar(
                        s[:], dT[h][:], pnsb[:, h : h + 1], None, op0=add
                    )
                    # --- 32x32 block transpose: k moves to the free dim.
                    # B[32P+i, 32F+j] = dists[n0+32F+i, 128h+32P+j]
                    bt = workp.tile([128, 512], F32, tag="B", name=f"B{g}_{h}")
                    nc.vector.transpose(bt[:], s[:])
                    b4 = bt.rearrange("p (f c2 k) -> p f c2 k", c2=2, k=K)
                    # --- min over k, one-hot (in block-transposed space)
                    mb = workp.tile([128, 32], F32, tag="minB", name=f"mB{g}_{h}")
                    nc.vector.tensor_reduce(
                        mb.rearrange("p (f c2) -> p f c2", c2=2), b4, axis=ax_x,
                        op=amin,
                    )
                    ohb = workp.tile([128, 512], F16, tag="ohB", name=f"oB{g}_{h}")
                    nc.vector.tensor_tensor(
                        ohb.rearrange("p (f c2 k) -> p f c2 k", c2=2, k=K),
                        b4,
                        mb.rearrange("p (f c2) -> p f c2", c2=2)[
                            :, :, :, None
                        ].broadcast_to([128, 16, 2, K]),
                        op=iseq,
                    )
                    # --- transpose back: otH[p, f] = onehot[n0+f, 128h+p]
                    o = workp.tile([128, 512], F16, tag="otH", name=f"oT{g}_{h}")
                    nc.vector.transpose(o[:], ohb[:])
                    otH.append(o)
                for tt in range(4):
                    t = 4 * g + tt
                    # --- LUT gather-accumulate: out[n, m] = onehot @ luts
                    ops = pout.tile([128, M], F32, tag="ops", name=f"ops{t}")
                    for h in range(2):
                        for mh in range(2):
                            nc.tensor.matmul(
                                ops[:, 512 * mh : 512 * (mh + 1)],
                                lhsT=otH[h][:, 128 * tt : 128 * (tt + 1)],
                                rhs=lutsb[
                                    :, M * h + 512 * mh : M * h + 512 * (mh + 1)
                                ],
                                start=(h == 0),
                                stop=(h == 1),
                            )
                    osb = osbp.tile([128, M], F32, tag="osb", name=f"osb{t}")
                    nc.scalar.copy(osb[:], ops[:])
                    nc.gpsimd.dma_start(out[128 * t : 128 * (t + 1), :], osb[:])

    nc.compile()
    return nc


_CACHE = {}


def _prep_shared(protos: np.ndarray, luts: np.ndarray, mean: float):
    wfull = np.zeros((D, CK), dtype=np.float32)
    for c in range(C):
        wfull[S * c : S * (c + 1), K * c : K * (c + 1)] = -2.0 * protos[c].T
    w4h = np.concatenate(
        [wfull[128 * b : 128 * (b + 1), 64 * b : 64 * (b + 1)] for b in range(4)],
        axis=1,
    )  # [128, 256] fp32
    whi = w4h.astype(np.float16)
    wlo = (w4h - whi.astype(np.float32)).astype(np.float16)
    w4hl = np.ascontiguousarray(np.concatenate([whi, wlo], axis=1))  # [128, 512]
    pnorm = (protos.astype(np.float64) ** 2).sum(-1).astype(np.float32).reshape(CK)
    pnt = np.ascontiguousarray(np.stack([pnorm[:128], pnorm[128:]], axis=1))
    lf = (luts.reshape(CK, M).astype(np.float64) * mean).astype(np.float32)
    luts2 = np.ascontiguousarray(
        np.concatenate([lf[:128], lf[128:]], axis=1)
    ).astype(np.float16)
    return w4hl, pnt, luts2


def kernel(X: np.ndarray, protos: np.ndarray, luts: np.ndarray) -> np.ndarray:
    X = np.asarray(X, dtype=np.float32)
    protos = np.asarray(protos, dtype=np.float32)
    luts = np.asarray(luts, dtype=np.float32)

    mean = float(np.mean(X, dtype=np.float64))
    w4hl, pnt, luts2 = _prep_shared(protos, luts, mean)

    if "nc" not in _CACHE:
        _CACHE["nc"] = _build_program()
    nc = _CACHE["nc"]

    in_maps = []
    for i in range(NCORES):
        xt = np.ascontiguousarray(X[NL * i : NL * (i + 1)].T)  # [D, NL] fp32
        xh = xt.astype(np.float16)
        xl = (xt - xh.astype(np.float32)).astype(np.float16)
        xhl = np.empty((D, NG, 1024), dtype=np.float16)
        xhl[:, :, :512] = xh.reshape(D, NG, 512)
        xhl[:, :, 512:] = xl.reshape(D, NG, 512)
        in_maps.append(
            {
                "xthl": xhl,
                "w4": w4hl,
                "pnt": pnt,
                "luts": luts2,
            }
        )

    trace = bool(os.environ.get("VQ_TRACE"))
    if trace:
        trace = _install_profile_shim()

    res = run_bass_kernel_spmd(nc, in_maps, core_ids=list(range(NCORES)), trace=trace)
    _CACHE["exec_time_ns"] = res.exec_time_ns
    _CACHE["profile_json"] = res.profile_json

    return np.concatenate([res.results[i]["out"] for i in range(NCORES)], axis=0)


# revision 18
# speedup vs baseline: 1.2502x; 1.0217x over previous
"""MithralNN (PQ/vq_codebook) Trainium2 kernel.

out[n, m] = sum_c luts[c, argmin_k ||X[n, cS:(c+1)S] - protos[c,k]||^2, m] * mean(X)

Strategy (8 NeuronCores, data-parallel over rows of X):
  host:   transpose each core's X shard to [D, NL] and split into fp16
          hi/lo halves (same total bytes as fp32, but PE streams fp16 at
          2x the rate); same hi/lo split for the block-diagonal -2*protos
          weights; ||p||^2 bias as a per-partition column; luts scaled by
          mean(X) and cast to fp16.
  device, per 512-row group:
          dists^T[ck, n] = sum of 3 fp16 hi/lo cross-product matmuls
          (= fp32-accurate dots; error ~2^-22)        (PE)
          += ||p||^2 via per-partition tensor_scalar add -> SBUF  (DVE)
          DVE 32x32 stream-transpose puts k on the free dim: segmented
          min-reduce + is_equal give the one-hot in block-transposed space,
          and a second stream-transpose yields onehot^T[ck, n] directly.
          out = onehot @ luts (PE, fp16), out copy on ScalarE, DMA out.
"""

import os
import sys
import types

import numpy as np

import concourse.bacc as bacc
import concourse.mybir as mybir
import concourse.tile as tile
from concourse.bass_utils import run_bass_kernel_spmd

F32 = mybir.dt.float32
F16 = mybir.dt.float16

N, D = 32768, 512
C, K, S = 16, 16, 32
CK = C * K          # 256
M = 1024
NCORES = 8
NL = N // NCORES    # 4096 rows per core
NT = NL // 128      # 32 row tiles per core
NG = NL // 512      # 8 chunks of 512 rows


def _install_profile_shim():
    try:
        if "antenv.axon_hooks" not in sys.modules:
            import antenv

            mod = types.ModuleType("antenv.axon_hooks")
            mod._hook = None
            mod.set_axon_ntff_profile_hook = lambda h: setattr(mod, "_hook", h)
            mod.get_axon_ntff_profile_hook = lambda: mod._hook
            sys.modules["antenv.axon_hooks"] = mod
            antenv.axon_hooks = mod
            from trn_agent_boot.trn_boot import _ntff_profile_via_ctypes

            mod.set_axon_ntff_profile_hook(
                _ntff_profile_via_ctypes("/opt/axon/libaxon_pjrt.so")
            )
        return True
    except Exception:
        return False


def _build_program():
    nc = bacc.Bacc("TRN2", target_bir_lowering=False, debug=False)

    xthl = nc.dram_tensor("xthl", [D, NG, 1024], F16, kind="ExternalInput")
    w4 = nc.dram_tensor("w4", [128, 512], F16, kind="ExternalInput")
    pnt = nc.dram_tensor("pnt", [128, 2], F32, kind="ExternalInput")
    luts = nc.dram_tensor("luts", [128, 2 * M], F16, kind="ExternalInput")
    out = nc.dram_tensor("out", [NL, M], F32, kind="ExternalOutput")

    add = mybir.AluOpType.add
    amin = mybir.AluOpType.min
    iseq = mybir.AluOpType.is_equal
    ax_x = mybir.AxisListType.X

    with tile.TileContext(nc) as tc:
        with (
            tc.tile_pool(name="const", bufs=1) as constp,
            tc.tile_pool(name="xch", bufs=3) as xpool,
            tc.tile_pool(name="work", bufs=4) as workp,
            tc.tile_pool(name="osb", bufs=3) as osbp,
            tc.tile_pool(name="pdT", bufs=4, space="PSUM") as pdT,
            tc.tile_pool(name="pout", bufs=2, space="PSUM") as pout,
        ):
            w4sb = constp.tile([128, 512], F16)
            nc.sync.dma_start(w4sb[:], w4[:])
            pnsb = constp.tile([128, 2], F32)
            nc.sync.dma_start(pnsb[:], pnt[:])
            lutsb = constp.tile([128, 2 * M], F16)
            nc.sync.dma_start(lutsb[:], luts[:])

            for g in range(NG):
                cj = xpool.tile([128, 4 * 1024], F16, tag="xhl", name=f"cj{g}")
                for b in range(4):
                    nc.sync.dma_start(
                        cj[:, 1024 * b : 1024 * (b + 1)],
                        xthl[128 * b : 128 * (b + 1), g, :],
                    )
                # --- encode: dists'^T[ck, n] = sum_d (-2W)[d,ck] * xt[d,n]
                # fp16 hi/lo cross products: Whi*Xhi + Whi*Xlo + Wlo*Xhi
                dT = [
                    pdT.tile([128, 512], F32, tag="dT", name=f"dT{g}_{h}")
                    for h in range(2)
                ]
                for b in range(4):
                    dst = dT[b // 2][64 * (b % 2) : 64 * (b % 2) + 64, :]
                    whi = w4sb[:, 64 * b : 64 * (b + 1)]
                    wlo = w4sb[:, 256 + 64 * b : 256 + 64 * (b + 1)]
                    xh_ = cj[:, 1024 * b : 1024 * b + 512]
                    xl_ = cj[:, 1024 * b + 512 : 1024 * b + 1024]
                    nc.tensor.matmul(dst, lhsT=whi, rhs=xh_, start=True, stop=False)
                    nc.tensor.matmul(dst, lhsT=whi, rhs=xl_, start=False, stop=False)
                    nc.tensor.matmul(dst, lhsT=wlo, rhs=xh_, start=False, stop=True)
                otH = []
                for h in range(2):
                    # --- += ||p||^2 (per-partition scalar), PSUM -> SBUF
                    s = workp.tile([128, 512], F32, tag="dstT", name=f"dstT{g}_{h}")
                    nc.vector.tensor_scalar(
                        s[:], dT[h][:], pnsb[:, h : h + 1], None, op0=add
                    )
                    # --- 32x32 block transpose: k moves to the free dim.
                    # B[32P+i, 32F+j] = dists[n0+32F+i, 128h+32P+j]
                    bt = workp.tile([128, 512], F32, tag="B", name=f"B{g}_{h}")
                    nc.vector.transpose(bt[:], s[:])
                    b4 = bt.rearrange("p (f c2 k) -> p f c2 k", c2=2, k=K)
                    # --- min over k, one-hot (in block-transposed space)
                    mb = workp.tile([128, 32], F32, tag="minB", name=f"mB{g}_{h}")
                    nc.vector.tensor_reduce(
                        mb.rearrange("p (f c2) -> p f c2", c2=2), b4, axis=ax_x,
                        op=amin,
                    )
                    ohb = workp.tile([128, 512], F16, tag="ohB", name=f"oB{g}_{h}")
                    nc.vector.tensor_tensor(
                        ohb.rearrange("p (f c2 k) -> p f c2 k", c2=2, k=K),
                        b4,
                        mb.rearrange("p (f c2) -> p f c2", c2=2)[
                            :, :, :, None
                        ].broadcast_to([128, 16, 2, K]),
                        op=iseq,
                    )
                    # --- transpose back: otH[p, f] = onehot[n0+f, 128h+p]
                    o = workp.tile([128, 512], F16, tag="otH", name=f"oT{g}_{h}")
                    nc.vector.transpose(o[:], ohb[:])
                    otH.append(o)
                for tt in range(4):
                    t = 4 * g + tt
                    # --- LUT gather-accumulate: out[n, m] = onehot @ luts
                    ops = pout.tile([128, M], F32, tag="ops", name=f"ops{t}")
                    for h in range(2):
                        for mh in range(2):
                            nc.tensor.matmul(
                                ops[:, 512 * mh : 512 * (mh + 1)],
                                lhsT=otH[h][:, 128 * tt : 128 * (tt + 1)],
                                rhs=lutsb[
                                    :, M * h + 512 * mh : M * h + 512 * (mh + 1)
                                ],
                                start=(h == 0),
                                stop=(h == 1),
                            )
                    osb = osbp.tile([128, M], F32, tag="osb", name=f"osb{t}")
                    nc.scalar.copy(osb[:], ops[:])
                    nc.gpsimd.dma_start(out[128 * t : 128 * (t + 1), :], osb[:])

    nc.compile()
    return nc


_CACHE = {}


def _prep_shared(protos: np.ndarray, luts: np.ndarray, mean: float):
    wfull = np.zeros((D, CK), dtype=np.float32)
    for c in range(C):
        wfull[S * c : S * (c + 1), K * c : K * (c + 1)] = -2.0 * protos[c].T
    w4h = np.concatenate(
        [wfull[128 * b : 128 * (b + 1), 64 * b : 64 * (b + 1)] for b in range(4)],
        axis=1,
    )  # [128, 256] fp32
    whi = w4h.astype(np.float16)
    wlo = (w4h - whi.astype(np.float32)).astype(np.float16)
    w4hl = np.ascontiguousarray(np.concatenate([whi, wlo], axis=1))  # [128, 512]
    pnorm = (protos.astype(np.float64) ** 2).sum(-1).astype(np.float32).reshape(CK)
    pnt = np.ascontiguousarray(np.stack([pnorm[:128], pnorm[128:]], axis=1))
    lf = (luts.reshape(CK, M).astype(np.float64) * mean).astype(np.float32)
    luts2 = np.ascontiguousarray(
        np.concatenate([lf[:128], lf[128:]], axis=1)
    ).astype(np.float16)
    return w4hl, pnt, luts2


def kernel(X: np.ndarray, protos: np.ndarray, luts: np.ndarray) -> np.ndarray:
    X = np.asarray(X, dtype=np.float32)
    protos = np.asarray(protos, dtype=np.float32)
    luts = np.asarray(luts, dtype=np.float32)

    mean = float(np.mean(X, dtype=np.float64))
    w4hl, pnt, luts2 = _prep_shared(protos, luts, mean)

    if "nc" not in _CACHE:
        _CACHE["nc"] = _build_program()
    nc = _CACHE["nc"]

    in_maps = []
    for i in range(NCORES):
        xt = np.ascontiguousarray(X[NL * i : NL * (i + 1)].T)  # [D, NL] fp32
        xh = xt.astype(np.float16)
        xl = (xt - xh.astype(np.float32)).astype(np.float16)
        xhl = np.empty((D, NG, 1024), dtype=np.float16)
        xhl[:, :, :512] = xh.reshape(D, NG, 512)
        xhl[:, :, 512:] = xl.reshape(D, NG, 512)
        in_maps.append(
            {
                "xthl": xhl,
                "w4": w4hl,
                "pnt": pnt,
                "luts": luts2,
            }
        )

    trace = bool(os.environ.get("VQ_TRACE"))
    if trace:
        trace = _install_profile_shim()

    res = run_bass_kernel_spmd(nc, in_maps, core_ids=list(range(NCORES)), trace=trace)
    _CACHE["exec_time_ns"] = res.exec_time_ns
    _CACHE["profile_json"] = res.profile_json

    return np.concatenate([res.results[i]["out"] for i in range(NCORES)], axis=0)


# revision 19
# speedup vs baseline: 1.2645x; 1.0114x over previous
"""MithralNN (PQ/vq_codebook) Trainium2 kernel.

out[n, m] = sum_c luts[c, argmin_k ||X[n, cS:(c+1)S] - protos[c,k]||^2, m] * mean(X)

Strategy (8 NeuronCores, data-parallel over rows of X):
  host:   transpose each core's X shard to [D, NL] and split into fp16
          hi/lo halves (same total bytes as fp32, but PE streams fp16 at
          2x the rate); same hi/lo split for the block-diagonal -2*protos
          weights; ||p||^2 bias as a per-partition column; luts scaled by
          mean(X) and cast to fp16.
  device, per 512-row group:
          dists^T[ck, n] = sum of 3 fp16 hi/lo cross-product matmuls
          (= fp32-accurate dots; error ~2^-22)        (PE)
          += ||p||^2 via per-partition tensor_scalar add -> SBUF  (DVE)
          DVE 32x32 stream-transpose puts k on the free dim: segmented
          min-reduce + is_equal give the one-hot in block-transposed space,
          and a second stream-transpose yields onehot^T[ck, n] directly.
          out = onehot @ luts (PE, fp16), out copy on ScalarE, DMA out.
"""

import os
import sys
import types

import numpy as np

import concourse.bacc as bacc
import concourse.mybir as mybir
import concourse.tile as tile
from concourse.bass_utils import run_bass_kernel_spmd

F32 = mybir.dt.float32
F16 = mybir.dt.float16

N, D = 32768, 512
C, K, S = 16, 16, 32
CK = C * K          # 256
M = 1024
NCORES = 8
NL = N // NCORES    # 4096 rows per core
NT = NL // 128      # 32 row tiles per core
NG = NL // 512      # 8 chunks of 512 rows


def _install_profile_shim():
    try:
        if "antenv.axon_hooks" not in sys.modules:
            import antenv

            mod = types.ModuleType("antenv.axon_hooks")
            mod._hook = None
            mod.set_axon_ntff_profile_hook = lambda h: setattr(mod, "_hook", h)
            mod.get_axon_ntff_profile_hook = lambda: mod._hook
            sys.modules["antenv.axon_hooks"] = mod
            antenv.axon_hooks = mod
            from trn_agent_boot.trn_boot import _ntff_profile_via_ctypes

            mod.set_axon_ntff_profile_hook(
                _ntff_profile_via_ctypes("/opt/axon/libaxon_pjrt.so")
            )
        return True
    except Exception:
        return False


def _build_program():
    nc = bacc.Bacc("TRN2", target_bir_lowering=False, debug=False)

    xthl = nc.dram_tensor("xthl", [D, NG, 1024], F16, kind="ExternalInput")
    w4 = nc.dram_tensor("w4", [128, 512], F16, kind="ExternalInput")
    pnt = nc.dram_tensor("pnt", [128, 2], F32, kind="ExternalInput")
    luts = nc.dram_tensor("luts", [128, 2 * M], F16, kind="ExternalInput")
    out = nc.dram_tensor("out", [NL, M], F32, kind="ExternalOutput")

    add = mybir.AluOpType.add
    amin = mybir.AluOpType.min
    iseq = mybir.AluOpType.is_equal
    ax_x = mybir.AxisListType.X

    with tile.TileContext(nc) as tc:
        with (
            tc.tile_pool(name="const", bufs=1) as constp,
            tc.tile_pool(name="xch", bufs=3) as xpool,
            tc.tile_pool(name="work", bufs=4) as workp,
            tc.tile_pool(name="osb", bufs=3) as osbp,
            tc.tile_pool(name="pdT", bufs=4, space="PSUM") as pdT,
            tc.tile_pool(name="pout", bufs=2, space="PSUM") as pout,
        ):
            w4sb = constp.tile([128, 512], F16)
            nc.sync.dma_start(w4sb[:], w4[:])
            pnsb = constp.tile([128, 2], F32)
            nc.sync.dma_start(pnsb[:], pnt[:])
            lutsb = constp.tile([128, 2 * M], F16)
            nc.gpsimd.dma_start(lutsb[:], luts[:])

            # HAM warmup: run PE during the initial DMA fill so the clock
            # gate opens before the first real matmul burst.
            warm = pdT.tile([128, 128], F32, tag="dT", name="warm")
            for _ in range(36):
                nc.tensor.matmul(
                    warm[:], lhsT=w4sb[:, :128], rhs=w4sb[:, :128],
                    start=True, stop=True,
                )

            for g in range(NG):
                cj = xpool.tile([128, 4 * 1024], F16, tag="xhl", name=f"cj{g}")
                for b in range(4):
                    nc.sync.dma_start(
                        cj[:, 1024 * b : 1024 * (b + 1)],
                        xthl[128 * b : 128 * (b + 1), g, :],
                    )
                # --- encode: dists'^T[ck, n] = sum_d (-2W)[d,ck] * xt[d,n]
                # fp16 hi/lo cross products: Whi*Xhi + Whi*Xlo + Wlo*Xhi
                dT = [
                    pdT.tile([128, 512], F32, tag="dT", name=f"dT{g}_{h}")
                    for h in range(2)
                ]
                for b in range(4):
                    dst = dT[b // 2][64 * (b % 2) : 64 * (b % 2) + 64, :]
                    whi = w4sb[:, 64 * b : 64 * (b + 1)]
                    wlo = w4sb[:, 256 + 64 * b : 256 + 64 * (b + 1)]
                    xh_ = cj[:, 1024 * b : 1024 * b + 512]
                    xl_ = cj[:, 1024 * b + 512 : 1024 * b + 1024]
                    nc.tensor.matmul(dst, lhsT=whi, rhs=xh_, start=True, stop=False)
                    nc.tensor.matmul(dst, lhsT=whi, rhs=xl_, start=False, stop=False)
                    nc.tensor.matmul(dst, lhsT=wlo, rhs=xh_, start=False, stop=True)
                otH = []
                for h in range(2):
                    # --- += ||p||^2 (per-partition scalar), PSUM -> SBUF
                    s = workp.tile([128, 512], F32, tag="dstT", name=f"dstT{g}_{h}")
                    nc.vector.tensor_scalar(
                        s[:], dT[h][:], pnsb[:, h : h + 1], None, op0=add
                    )
                    # --- 32x32 block transpose: k moves to the free dim.
                    # B[32P+i, 32F+j] = dists[n0+32F+i, 128h+32P+j]
                    bt = workp.tile([128, 512], F32, tag="B", name=f"B{g}_{h}")
                    nc.vector.transpose(bt[:], s[:])
                    b4 = bt.rearrange("p (f c2 k) -> p f c2 k", c2=2, k=K)
                    # --- min over k, one-hot (in block-transposed space)
                    mb = workp.tile([128, 32], F32, tag="minB", name=f"mB{g}_{h}")
                    nc.vector.tensor_reduce(
                        mb.rearrange("p (f c2) -> p f c2", c2=2), b4, axis=ax_x,
                        op=amin,
                    )
                    ohb = workp.tile([128, 512], F16, tag="ohB", name=f"oB{g}_{h}")
                    nc.vector.tensor_tensor(
                        ohb.rearrange("p (f c2 k) -> p f c2 k", c2=2, k=K),
                        b4,
                        mb.rearrange("p (f c2) -> p f c2", c2=2)[
                            :, :, :, None
                        ].broadcast_to([128, 16, 2, K]),
                        op=iseq,
                    )
                    # --- transpose back: otH[p, f] = onehot[n0+f, 128h+p]
                    o = workp.tile([128, 512], F16, tag="otH", name=f"oT{g}_{h}")
                    nc.vector.transpose(o[:], ohb[:])
                    otH.append(o)
                for tt in range(4):
                    t = 4 * g + tt
                    # --- LUT gather-accumulate: out[n, m] = onehot @ luts
                    ops = pout.tile([128, M], F32, tag="ops", name=f"ops{t}")
                    for h in range(2):
                        for mh in range(2):
                            nc.tensor.matmul(
                                ops[:, 512 * mh : 512 * (mh + 1)],
                                lhsT=otH[h][:, 128 * tt : 128 * (tt + 1)],
                                rhs=lutsb[
                                    :, M * h + 512 * mh : M * h + 512 * (mh + 1)
                                ],
                                start=(h == 0),
                                stop=(h == 1),
                            )
                    osb = osbp.tile([128, M], F32, tag="osb", name=f"osb{t}")
                    nc.scalar.copy(osb[:], ops[:])
                    nc.gpsimd.dma_start(out[128 * t : 128 * (t + 1), :], osb[:])

    nc.compile()
    return nc


_CACHE = {}


def _prep_shared(protos: np.ndarray, luts: np.ndarray, mean: float):
    wfull = np.zeros((D, CK), dtype=np.float32)
    for c in range(C):
        wfull[S * c : S * (c + 1), K * c : K * (c + 1)] = -2.0 * protos[c].T
    w4h = np.concatenate(
        [wfull[128 * b : 128 * (b + 1), 64 * b : 64 * (b + 1)] for b in range(4)],
        axis=1,
    )  # [128, 256] fp32
    whi = w4h.astype(np.float16)
    wlo = (w4h - whi.astype(np.float32)).astype(np.float16)
    w4hl = np.ascontiguousarray(np.concatenate([whi, wlo], axis=1))  # [128, 512]
    pnorm = (protos.astype(np.float64) ** 2).sum(-1).astype(np.float32).reshape(CK)
    pnt = np.ascontiguousarray(np.stack([pnorm[:128], pnorm[128:]], axis=1))
    lf = (luts.reshape(CK, M).astype(np.float64) * mean).astype(np.float32)
    luts2 = np.ascontiguousarray(
        np.concatenate([lf[:128], lf[128:]], axis=1)
    ).astype(np.float16)
    return w4hl, pnt, luts2


def kernel(X: np.ndarray, protos: np.ndarray, luts: np.ndarray) -> np.ndarray:
    X = np.asarray(X, dtype=np.float32)
    protos = np.asarray(protos, dtype=np.float32)
    luts = np.asarray(luts, dtype=np.float32)

    mean = float(np.mean(X, dtype=np.float64))
    w4hl, pnt, luts2 = _prep_shared(protos, luts, mean)

    if "nc" not in _CACHE:
        _CACHE["nc"] = _build_program()
    nc = _CACHE["nc"]

    in_maps = []
    for i in range(NCORES):
        xt = np.ascontiguousarray(X[NL * i : NL * (i + 1)].T)  # [D, NL] fp32
        xh = xt.astype(np.float16)
        xl = (xt - xh.astype(np.float32)).astype(np.float16)
        xhl = np.empty((D, NG, 1024), dtype=np.float16)
        xhl[:, :, :512] = xh.reshape(D, NG, 512)
        xhl[:, :, 512:] = xl.reshape(D, NG, 512)
        in_maps.append(
            {
                "xthl": xhl,
                "w4": w4hl,
                "pnt": pnt,
                "luts": luts2,
            }
        )

    trace = bool(os.environ.get("VQ_TRACE"))
    if trace:
        trace = _install_profile_shim()

    res = run_bass_kernel_spmd(nc, in_maps, core_ids=list(range(NCORES)), trace=trace)
    _CACHE["exec_time_ns"] = res.exec_time_ns
    _CACHE["profile_json"] = res.profile_json

    return np.concatenate([res.results[i]["out"] for i in range(NCORES)], axis=0)
